# revision 7
# baseline (speedup 1.0000x reference)
"""Adaptive per-pixel LoG 9x9 convolution on 8 TRN2 NeuronCores.

Math: out[b,c,y,x] = sum_{dy,dx in [-4,4]} xpad[b,c,y+dy,x+dx] * K(dx^2+dy^2; p)
The kernel depends on the offset only through r2 = dx^2+dy^2, which takes 15
distinct values over the 9x9 window -> exact rank-15 decomposition:
    out[c,p] = sum_v w_v[p] * S_v[c,p]
where S_v = fixed "ring sum" convolutions (shifted adds, shared through
column-class partial sums U_d and symmetric row-pair sums D) and
    w_v[p] = (base[p] - r2_v * B2[p]) * exp(-r2_v * inv2s2[p])
with base = -dist*sqrt(sigma)/(pi*sigma^4), inv2s2 = 1/(2 sigma^2),
B2 = base*inv2s2 (tiny per-pixel scalar fields, prepared on host).

Sharding: 8 cores = 4 batches x 2 row-halves (128 output rows each). Within a
core, SBUF partition p = (row-strip si, col-block bi): a 16x16 output tile plus
4-pixel halo (24x24 input window, all 3 channels). All shifts become free-dim
AP offsets; host bakes the replicated window layout so every DMA is contiguous.

Perf notes: everything heavy runs in bf16 (DVE 2x_1p); a 1-col-shifted input
copy (xp1) keeps every tap 4B-aligned.  Stage-2 row-pair sums are batched over
the 4 column classes per shift (one op per k).  Products use zero-stride
channel broadcast of the per-pixel weights; the 15 products are tree-reduced.
Input DMAs are spread over three engine queues; a dummy Exp pre-triggers the
ACT table load; output ships as bf16 and is upcast on host.
"""

import math

import numpy as np

B, C, H, W = 4, 3, 256, 256
PAD = 4
SIGMA_MIN, SIGMA_MAX = 0.5, 10.0
N_CORES = 8

S_ROWS = 16
S_COLS = 16
N_STRIPS = 8
N_BLOCKS = 16
IN_R = S_ROWS + 2 * PAD  # 24
IN_C = S_COLS + 2 * PAD  # 24
IN_C1 = 22  # xp1 = cols +1..+22 of the window (odd-shift taps, 4B-aligned)

R2_VALUES = sorted({dx * dx + dy * dy for dx in range(-4, 5) for dy in range(-4, 5)})
assert len(R2_VALUES) == 15
NV = 15
# physical ring order: S-resident slices 1..10, then the 4 D-diagonal rings
V_SRES = [1, 4, 9, 16, 5, 10, 13, 17, 20, 25]
V_DIAG = [2, 8, 18, 32]
V_ORD = V_SRES + V_DIAG  # Wv slot i / E slot i <-> ring V_ORD[i]


def _build_program(nc, bass, mybir):
    f32 = mybir.dt.float32
    bf16 = mybir.dt.bfloat16
    Alu = mybir.AluOpType
    Act = mybir.ActivationFunctionType

    xp_d = nc.declare_dram_parameter("xp", [128, C, IN_R, IN_C], bf16, isOutput=False)
    xp1_d = nc.declare_dram_parameter("xp1", [128, C, IN_R, IN_C1], bf16, isOutput=False)
    wb_d = nc.declare_dram_parameter("wb", [128, 2, S_ROWS, S_COLS], bf16, isOutput=False)
    inv_d = nc.declare_dram_parameter("inv", [128, S_ROWS, S_COLS], f32, isOutput=False)
    out_d = nc.declare_dram_parameter("out", [128, C, S_ROWS, S_COLS], bf16, isOutput=True)

    with (
        nc.Block() as block,
        nc.semaphore("dma_sem") as dma_sem,
        nc.semaphore("x1_sem") as x1_sem,
        nc.semaphore("win_sem") as win_sem,
        nc.semaphore("act_sem") as act_sem,
        nc.semaphore("dve_sem") as dve_sem,
        nc.sbuf_tensor("s_xp", [128, C, IN_R, IN_C], bf16) as xp,
        nc.sbuf_tensor("s_xp1", [128, C, IN_R, IN_C1], bf16) as xp1,
        nc.sbuf_tensor("s_wb", [128, 2, S_ROWS, S_COLS], bf16) as wb,
        nc.sbuf_tensor("s_inv", [128, S_ROWS, S_COLS], f32) as inv,
        nc.sbuf_tensor("U", [128, 4, C, IN_R, S_COLS], bf16) as U,
        nc.sbuf_tensor("D", [128, 4, 4, C, S_ROWS, S_COLS], bf16) as D,
        nc.sbuf_tensor("D0", [128, 4, C, S_ROWS, S_COLS], bf16) as D0,
        nc.sbuf_tensor("S", [128, 11, C, S_ROWS, S_COLS], bf16) as S,
        nc.sbuf_tensor("E", [128, 14, S_ROWS, S_COLS], bf16) as E,
        nc.sbuf_tensor("Wv", [128, 14, S_ROWS, S_COLS], bf16) as Wv,
        nc.sbuf_tensor("P", [128, NV, C, S_ROWS, S_COLS], bf16) as P,
        nc.sbuf_tensor("scr", [128, 2], f32) as scr,
    ):
        base_b = wb[:, 0]
        b2_b = wb[:, 1]

        @block.sync
        def _(sync):
            sync.dma_start(out=inv[:], in_=inv_d[:]).then_inc(dma_sem, 16)
            sync.dma_start(out=xp[:], in_=xp_d[:]).then_inc(dma_sem, 16)
            sync.wait_ge(dve_sem, 1)
            sync.dma_start(out=out_d[:], in_=P[:, 0]).then_inc(dma_sem, 16)
            sync.wait_ge(dma_sem, 48)

        @block.gpsimd
        def _(gpsimd):
            gpsimd.dma_start(out=xp1[:], in_=xp1_d[:]).then_inc(x1_sem, 16)

        @block.scalar
        def _(scalar):
            scalar.dma_start(out=wb[:], in_=wb_d[:]).then_inc(win_sem, 16)
            # dummy exp: pulls the ACT table load off the critical path
            scalar.activation(scr[:, 0:1], scr[:, 1:2], Act.Exp, bias=0.0, scale=0.0)
            scalar.wait_ge(dma_sem, 16)
            for i, v in enumerate(V_ORD):
                scalar.activation(
                    E[:, i], inv[:], Act.Exp, bias=0.0, scale=float(-v)
                ).then_inc(act_sem, 1)

        @block.vector
        def _(vector):
            # w-gen: Wv[i] = base - r2_v * B2   (TS at 4x, then TT at 2x)
            vector.wait_ge(win_sem, 16)
            for i, v in enumerate(V_ORD):
                vector.tensor_scalar(Wv[:, i], b2_b, float(-v), None, Alu.mult)
            for i in range(14):
                vector.tensor_tensor(Wv[:, i], Wv[:, i], base_b, Alu.add)

            # stage 1: column-class sums U_d over all strip rows
            vector.wait_ge(dma_sem, 32)
            vector.wait_ge(x1_sem, 16)
            vector.tensor_tensor(
                U[:, 0], xp1[:, :, :, 2 : 2 + S_COLS], xp1[:, :, :, 4 : 4 + S_COLS], Alu.add
            )
            vector.tensor_tensor(
                U[:, 1], xp[:, :, :, 2 : 2 + S_COLS], xp[:, :, :, 6 : 6 + S_COLS], Alu.add
            )
            vector.tensor_tensor(
                U[:, 2], xp1[:, :, :, 0:S_COLS], xp1[:, :, :, 6 : 6 + S_COLS], Alu.add
            )
            vector.tensor_tensor(
                U[:, 3], xp[:, :, :, 0:S_COLS], xp[:, :, :, 8 : 8 + S_COLS], Alu.add
            )

            # stage 2a: symmetric row-pair sums, batched over the 4 col classes
            # D[k-1, a-1] = U_a[rows 4-k ..] + U_a[rows 4+k ..]
            for k in range(1, 5):
                vector.tensor_tensor(
                    D[:, k - 1],
                    U[:, :, :, PAD - k : PAD - k + S_ROWS, :],
                    U[:, :, :, PAD + k : PAD + k + S_ROWS, :],
                    Alu.add,
                )
            # D0[k-1] = center-column pairs from xp
            for k in range(1, 5):
                vector.tensor_tensor(
                    D0[:, k - 1],
                    xp[:, :, PAD - k : PAD - k + S_ROWS, PAD : PAD + S_COLS],
                    xp[:, :, PAD + k : PAD + k + S_ROWS, PAD : PAD + S_COLS],
                    Alu.add,
                )
            # stage 2b: ring assembly
            # centers (v = 1,4,9,16 -> S slices 1..4): U_a[dy=0] + D0[k=a]
            vector.tensor_tensor(
                S[:, 1:5], U[:, :, :, PAD : PAD + S_ROWS, :], D0[:, :], Alu.add
            )
            # mixed pairs -> S slices 5..10 (v = 5,10,13,17,20,25)
            for si, (k1, a1, k2, a2) in zip(
                range(5, 11),
                [(1, 2, 2, 1), (1, 3, 3, 1), (2, 3, 3, 2), (1, 4, 4, 1), (2, 4, 4, 2), (3, 4, 4, 3)],
            ):
                vector.tensor_tensor(
                    S[:, si], D[:, k1 - 1, a1 - 1], D[:, k2 - 1, a2 - 1], Alu.add
                )

            # w-gen part 2: Wv[i] *= E[i]
            vector.wait_ge(act_sem, 14)
            for i in range(14):
                vector.tensor_tensor(Wv[:, i], Wv[:, i], E[:, i], Alu.mult)

            def bcast(src, lead=()):
                # insert extra leading dims + a 0-stride channel dim into a
                # [128, 16, 16] view: -> [128, *lead, C(0-stride), 16, 16]
                return bass.AP(
                    src.tensor,
                    src.offset,
                    [list(src.ap[0])]
                    + [list(d) for d in lead]
                    + [[0, C]]
                    + [list(d) for d in src.ap[-2:]],
                )

            # products
            # P[0] = S0 (xp center view) * base
            vector.tensor_tensor(
                P[:, 0],
                xp[:, :, PAD : PAD + S_ROWS, PAD : PAD + S_COLS],
                bcast(base_b),
                Alu.mult,
            )
            # P[1:11] = S[1:11] * Wv[0:10]  (one op, v+channel batched)
            vector.tensor_tensor(
                P[:, 1:11],
                S[:, 1:11],
                bcast(Wv[:, 0], lead=[[S_ROWS * S_COLS, 10]]),
                Alu.mult,
            )
            # P[11+j] = D[j,j] * Wv[10+j]   (diagonal rings v = 2,8,18,32)
            for j in range(4):
                vector.tensor_tensor(
                    P[:, 11 + j], D[:, j, j], bcast(Wv[:, 10 + j]), Alu.mult
                )

            # tree-reduce the 15 products
            vector.tensor_tensor(P[:, 0:7], P[:, 0:7], P[:, 7:14], Alu.add)
            vector.tensor_tensor(P[:, 0:3], P[:, 0:3], P[:, 3:6], Alu.add)
            vector.tensor_tensor(P[:, 0], P[:, 0], P[:, 1], Alu.add)
            vector.tensor_tensor(P[:, 0], P[:, 0], P[:, 2], Alu.add)
            vector.tensor_tensor(P[:, 0], P[:, 0], P[:, 6], Alu.add)
            vector.tensor_tensor(P[:, 0], P[:, 0], P[:, 14], Alu.add).then_inc(dve_sem, 1)

    return nc


_PROGRAM_CACHE = {}


def _get_program():
    if "nc" not in _PROGRAM_CACHE:
        import sys

        if "/opt/trn_rl_repo" not in sys.path:
            sys.path.insert(0, "/opt/trn_rl_repo")
        from concourse import bass, mybir

        nc = bass.Bass()
        _PROGRAM_CACHE["nc"] = _build_program(nc, bass, mybir)
    return _PROGRAM_CACHE["nc"]


def _host_prep(x, foa_xy):
    import ml_dtypes

    bf = ml_dtypes.bfloat16
    xpad = np.pad(x, ((0, 0), (0, 0), (PAD, PAD), (PAD, PAD)), mode="reflect")
    xpad_bf = xpad.astype(bf)
    diag = math.sqrt(H * H + W * W)
    in_maps = []
    for core in range(N_CORES):
        b, half = divmod(core, 2)
        y0 = half * 128
        xph = xpad_bf[b, :, y0 : y0 + 136, :]  # [3, 136, 264]
        sw = np.lib.stride_tricks.sliding_window_view(xph, (C, IN_R, IN_C))
        XP = sw[0, ::S_ROWS, ::S_COLS]
        XP = np.ascontiguousarray(XP.reshape(128, C, IN_R, IN_C))
        sw1 = np.lib.stride_tricks.sliding_window_view(xph, (C, IN_R, IN_C1))
        XP1 = sw1[0, ::S_ROWS, 1::S_COLS][:, :N_BLOCKS]
        XP1 = np.ascontiguousarray(XP1.reshape(128, C, IN_R, IN_C1))

        yy, xx = np.meshgrid(
            np.arange(y0, y0 + 128, dtype=np.float64),
            np.arange(W, dtype=np.float64),
            indexing="ij",
        )
        fx, fy = float(foa_xy[b, 0]), float(foa_xy[b, 1])
        dist = np.sqrt((xx - fx) ** 2 + (yy - fy) ** 2)
        dn = dist / diag
        sigma = (1.0 - dn) * SIGMA_MIN + dn * SIGMA_MAX
        inv2s2 = 1.0 / (2.0 * sigma * sigma)
        base = -dist * np.sqrt(sigma) / (math.pi * sigma**4)
        b2 = base * inv2s2

        def tiles(a, dtype):
            t = a.reshape(N_STRIPS, S_ROWS, N_BLOCKS, S_COLS)
            return np.ascontiguousarray(
                t.transpose(0, 2, 1, 3).reshape(128, S_ROWS, S_COLS).astype(dtype)
            )

        WB = np.stack([tiles(base, bf), tiles(b2, bf)], axis=1)  # [128,2,16,16]
        INV = tiles(inv2s2, np.float32)

        in_maps.append(
            {"xp": XP, "xp1": XP1, "wb": np.ascontiguousarray(WB), "inv": INV}
        )
    return in_maps


def _gather(results):
    out = np.empty((B, C, H, W), dtype=np.float32)
    for core in range(N_CORES):
        b, half = divmod(core, 2)
        y0 = half * 128
        o = results[core]["out"].astype(np.float32)
        o = o.reshape(N_STRIPS, N_BLOCKS, C, S_ROWS, S_COLS)
        o = o.transpose(2, 0, 3, 1, 4).reshape(C, 128, W)
        out[b, :, y0 : y0 + 128, :] = o
    return out


def kernel(x, foa_xy, _trace=False, _tmpdir=None):
    import sys

    if "/opt/trn_rl_repo" not in sys.path:
        sys.path.insert(0, "/opt/trn_rl_repo")
    from concourse.bass_utils import run_bass_kernel_spmd

    nc = _get_program()
    in_maps = _host_prep(np.asarray(x), np.asarray(foa_xy))
    kw = {}
    if _trace:
        kw = dict(trace=True, trace_cores=[], tmpdir=_tmpdir)
    res = run_bass_kernel_spmd(nc, in_maps, list(range(N_CORES)), **kw)
    out = _gather(res.results)
    if _trace:
        return out, res
    return out


# revision 12
# speedup vs baseline: 1.2118x; 1.2118x over previous
"""Adaptive per-pixel LoG 9x9 convolution on 8 TRN2 NeuronCores.

out[b,c,y,x] = sum_{dy,dx in [-4,4]} xpad[b,c,y+dy,x+dx] * K(dx^2+dy^2; p)
K depends on the offset only through r2 = dx^2+dy^2 (15 distinct values)
-> exact rank-15 decomposition  out[c,p] = sum_v w_v[p] * S_v[c,p]  where
S_v are fixed ring-sum convolutions (shared shifted adds) and
w_v = (base - r2_v*B2) * exp(-r2_v * inv2s2); base/B2/inv2s2 are smooth
per-pixel scalar fields from the focus-of-attention distance (host-prepared,
the exp lives on the scalar engine).

Sharding: 8 cores = 4 batches x 2 row-halves. Partition p = 16x16 output tile
+ 4px halo (24x24 window, 3 channels); all taps are free-dim AP offsets; host
bakes the window layout so DMAs are contiguous.

Perf: bf16 on the DVE (2x_1p; xp1 = 1-col-shifted copy keeps taps 4B-aligned).
HW cost is ~200cyc/op + 0.6cyc/elem -> everything is batched into few wide ops:
4 row-pair ops cover 16 ring partial sums, one op multiplies all 14 weight
planes by the exps, one mega-op forms 10 of the 15 products (zero-stride
channel broadcast), 5-op tree reduction. Critical input DMAs ride the two
HWDGE queues (sync+scalar, split in half); slow SWDGE (gpsimd) carries the
non-critical weight planes. Output ships bf16, split across three queues.
"""

import math

import numpy as np

B, C, H, W = 4, 3, 256, 256
PAD = 4
SIGMA_MIN, SIGMA_MAX = 0.5, 10.0
N_CORES = 8

S_ROWS = 16
S_COLS = 16
N_STRIPS = 8
N_BLOCKS = 16
IN_R = 24
IN_C = 24
IN_C1 = 22

R2_VALUES = sorted({dx * dx + dy * dy for dx in range(-4, 5) for dy in range(-4, 5)})
assert len(R2_VALUES) == 15
NV = 15
# ring order: slot 0 = r2=0; S-resident rings 1..10; D-diagonal rings 11..14
V_ORD = [1, 4, 9, 16, 5, 10, 17, 13, 20, 25, 2, 8, 18, 32]


def _build_program(nc, bass, mybir):
    f32 = mybir.dt.float32
    bf16 = mybir.dt.bfloat16
    Alu = mybir.AluOpType
    Act = mybir.ActivationFunctionType

    xp_d = nc.declare_dram_parameter("xp", [128, C, IN_R, IN_C], bf16, isOutput=False)
    xp1_d = nc.declare_dram_parameter("xp1", [128, C, IN_R, IN_C1], bf16, isOutput=False)
    wv_d = nc.declare_dram_parameter("wv", [128, NV, S_ROWS, S_COLS], bf16, isOutput=False)
    inv_d = nc.declare_dram_parameter("inv", [128, S_ROWS, S_COLS], f32, isOutput=False)
    out_d = nc.declare_dram_parameter("out", [128, C, S_ROWS, S_COLS], bf16, isOutput=True)

    with (
        nc.Block() as block,
        nc.semaphore("xa_sem") as xa_sem,
        nc.semaphore("x1a_sem") as x1a_sem,
        nc.semaphore("xb_sem") as xb_sem,
        nc.semaphore("x1b_sem") as x1b_sem,
        nc.semaphore("inv_sem") as inv_sem,
        nc.semaphore("wv_sem") as wv_sem,
        nc.semaphore("act_sem") as act_sem,
        nc.semaphore("dve_sem") as dve_sem,
        nc.semaphore("od_sem") as od_sem,
        nc.sbuf_tensor("s_xp", [128, C, IN_R, IN_C], bf16) as xp,
        nc.sbuf_tensor("s_xp1", [128, C, IN_R, IN_C1], bf16) as xp1,
        nc.sbuf_tensor("s_wv", [128, NV, S_ROWS, S_COLS], bf16) as Wv,
        nc.sbuf_tensor("s_inv", [128, S_ROWS, S_COLS], f32) as inv,
        nc.sbuf_tensor("U", [128, 4, C, IN_R, S_COLS], bf16) as U,
        nc.sbuf_tensor("D", [128, 4, 4, C, S_ROWS, S_COLS], bf16) as D,
        nc.sbuf_tensor("D0", [128, 4, C, S_ROWS, S_COLS], bf16) as D0,
        nc.sbuf_tensor("S", [128, 11, C, S_ROWS, S_COLS], bf16) as S,
        nc.sbuf_tensor("E", [128, 14, S_ROWS, S_COLS], bf16) as E,
        nc.sbuf_tensor("P", [128, NV, C, S_ROWS, S_COLS], bf16) as P,
        nc.sbuf_tensor("scr", [128, 2], f32) as scr,
    ):
        @block.sync
        def _(sync):
            sync.dma_start(out=xp[0:64], in_=xp_d[0:64]).then_inc(xa_sem, 16)
            sync.dma_start(out=xp1[0:64], in_=xp1_d[0:64]).then_inc(x1a_sem, 16)
            sync.wait_ge(dve_sem, 1)
            sync.dma_start(out=out_d[:, 0], in_=P[:, 0, 0]).then_inc(od_sem, 16)
            sync.wait_ge(od_sem, 48)

        @block.gpsimd
        def _(gpsimd):
            gpsimd.dma_start(out=inv[:], in_=inv_d[:]).then_inc(inv_sem, 16)
            gpsimd.dma_start(out=Wv[:], in_=wv_d[:]).then_inc(wv_sem, 16)
            gpsimd.wait_ge(dve_sem, 1)
            gpsimd.dma_start(out=out_d[:, 2], in_=P[:, 0, 2]).then_inc(od_sem, 16)
            gpsimd.wait_ge(od_sem, 48)

        @block.scalar
        def _(scalar):
            scalar.dma_start(out=xp[64:128], in_=xp_d[64:128]).then_inc(xb_sem, 16)
            scalar.dma_start(out=xp1[64:128], in_=xp1_d[64:128]).then_inc(x1b_sem, 16)
            # dummy exp pre-triggers the ACT table load
            scalar.activation(scr[:, 0:1], scr[:, 1:2], Act.Exp, bias=0.0, scale=0.0)
            scalar.wait_ge(inv_sem, 16)
            for i, v in enumerate(V_ORD):
                scalar.activation(
                    E[:, i], inv[:], Act.Exp, bias=0.0, scale=float(-v)
                ).then_inc(act_sem, 1)
            scalar.wait_ge(dve_sem, 1)
            scalar.dma_start(out=out_d[:, 1], in_=P[:, 0, 1]).then_inc(od_sem, 16)
            scalar.wait_ge(od_sem, 48)

        @block.vector
        def _(vector):
            # stage 1: column-class sums U_d over all strip rows
            vector.wait_ge(xa_sem, 16)
            vector.wait_ge(xb_sem, 16)
            vector.tensor_tensor(
                U[:, 1], xp[:, :, :, 2 : 2 + S_COLS], xp[:, :, :, 6 : 6 + S_COLS], Alu.add
            )
            vector.tensor_tensor(
                U[:, 3], xp[:, :, :, 0:S_COLS], xp[:, :, :, 8 : 8 + S_COLS], Alu.add
            )
            vector.wait_ge(x1a_sem, 16)
            vector.wait_ge(x1b_sem, 16)
            vector.tensor_tensor(
                U[:, 0], xp1[:, :, :, 2 : 2 + S_COLS], xp1[:, :, :, 4 : 4 + S_COLS], Alu.add
            )
            vector.tensor_tensor(
                U[:, 2], xp1[:, :, :, 0:S_COLS], xp1[:, :, :, 6 : 6 + S_COLS], Alu.add
            )

            # stage 2a: symmetric row-pair sums, batched over all 4 col classes
            for k in range(1, 5):
                vector.tensor_tensor(
                    D[:, k - 1],
                    U[:, :, :, PAD - k : PAD - k + S_ROWS, :],
                    U[:, :, :, PAD + k : PAD + k + S_ROWS, :],
                    Alu.add,
                )
            for k in range(1, 5):
                vector.tensor_tensor(
                    D0[:, k - 1],
                    xp[:, :, PAD - k : PAD - k + S_ROWS, PAD : PAD + S_COLS],
                    xp[:, :, PAD + k : PAD + k + S_ROWS, PAD : PAD + S_COLS],
                    Alu.add,
                )
            # stage 2b: ring assembly
            # centers: S[1..4] = U_a[dy=0] + D0[k=a]  (v = 1,4,9,16)
            vector.tensor_tensor(
                S[:, 1:5], U[:, :, :, PAD : PAD + S_ROWS, :], D0[:, :], Alu.add
            )
            def dview(k, a, n, stride):
                # n consecutive D[.] slices stepping by `stride` elements
                src = D[:, k, a]
                return bass.AP(
                    D,
                    src.offset,
                    [list(src.ap[0]), [stride, n]] + [list(x) for x in src.ap[1:]],
                )

            # mixed pairs: S[5..7] = D[1,{2,3,4}] + D[{2,3,4},1]  (v = 5,10,17)
            vector.tensor_tensor(
                S[:, 5:8], dview(0, 1, 3, 768), dview(1, 0, 3, 3072), Alu.add
            )
            # S[8..9] = D[2,{3,4}] + D[{3,4},2]  (v = 13,20)
            vector.tensor_tensor(
                S[:, 8:10], dview(1, 2, 2, 768), dview(2, 1, 2, 3072), Alu.add
            )
            # S[10] = D[3,4] + D[4,3]  (v = 25)
            vector.tensor_tensor(S[:, 10], D[:, 2, 3], D[:, 3, 2], Alu.add)

            # w-gen: Wv[1:15] *= E  (one wide op; host supplied base - r2*B2)
            vector.wait_ge(wv_sem, 16)
            vector.wait_ge(act_sem, 14)
            vector.tensor_tensor(Wv[:, 1:15], Wv[:, 1:15], E[:], Alu.mult)

            def bcast(src, lead=()):
                return bass.AP(
                    src.tensor,
                    src.offset,
                    [list(src.ap[0])]
                    + [list(d) for d in lead]
                    + [[0, C]]
                    + [list(d) for d in src.ap[-2:]],
                )

            # products
            vector.tensor_tensor(
                P[:, 0],
                xp[:, :, PAD : PAD + S_ROWS, PAD : PAD + S_COLS],
                bcast(Wv[:, 0]),
                Alu.mult,
            )
            vector.tensor_tensor(
                P[:, 1:11],
                S[:, 1:11],
                bcast(Wv[:, 1], lead=[[S_ROWS * S_COLS, 10]]),
                Alu.mult,
            )
            # P[11..14] = D[j,j] * Wv[11+j]   (v = 2,8,18,32)
            vector.tensor_tensor(
                P[:, 11:15],
                dview(0, 0, 4, 3840),
                bcast(Wv[:, 11], lead=[[S_ROWS * S_COLS, 4]]),
                Alu.mult,
            )

            # tree-reduce the 15 products (5 ops)
            vector.tensor_tensor(P[:, 0:7], P[:, 0:7], P[:, 7:14], Alu.add)
            vector.tensor_tensor(P[:, 0:3], P[:, 0:3], P[:, 3:6], Alu.add)
            # P[0]+=P[2], P[1]+=P[6] in one op
            p2 = P[:, 2]
            vector.tensor_tensor(
                P[:, 0:2],
                P[:, 0:2],
                bass.AP(
                    P,
                    p2.offset,
                    [list(p2.ap[0]), [4 * 768, 2]] + [list(x) for x in p2.ap[1:]],
                ),
                Alu.add,
            )
            vector.tensor_tensor(P[:, 0], P[:, 0], P[:, 1], Alu.add)
            vector.tensor_tensor(P[:, 0], P[:, 0], P[:, 14], Alu.add).then_inc(dve_sem, 3)

    return nc


_PROGRAM_CACHE = {}


def _get_program():
    if "nc" not in _PROGRAM_CACHE:
        import sys

        if "/opt/trn_rl_repo" not in sys.path:
            sys.path.insert(0, "/opt/trn_rl_repo")
        from concourse import bass, mybir

        nc = bass.Bass()
        _PROGRAM_CACHE["nc"] = _build_program(nc, bass, mybir)
    return _PROGRAM_CACHE["nc"]


def _host_prep(x, foa_xy):
    import ml_dtypes

    bf = ml_dtypes.bfloat16
    xpad = np.pad(x, ((0, 0), (0, 0), (PAD, PAD), (PAD, PAD)), mode="reflect")
    xpad_bf = xpad.astype(bf)
    diag = math.sqrt(H * H + W * W)
    in_maps = []
    for core in range(N_CORES):
        b, half = divmod(core, 2)
        y0 = half * 128
        xph = xpad_bf[b, :, y0 : y0 + 136, :]
        sw = np.lib.stride_tricks.sliding_window_view(xph, (C, IN_R, IN_C))
        XP = np.ascontiguousarray(sw[0, ::S_ROWS, ::S_COLS].reshape(128, C, IN_R, IN_C))
        sw1 = np.lib.stride_tricks.sliding_window_view(xph, (C, IN_R, IN_C1))
        XP1 = np.ascontiguousarray(
            sw1[0, ::S_ROWS, 1::S_COLS][:, :N_BLOCKS].reshape(128, C, IN_R, IN_C1)
        )

        yy, xx = np.meshgrid(
            np.arange(y0, y0 + 128, dtype=np.float64),
            np.arange(W, dtype=np.float64),
            indexing="ij",
        )
        fx, fy = float(foa_xy[b, 0]), float(foa_xy[b, 1])
        dist = np.sqrt((xx - fx) ** 2 + (yy - fy) ** 2)
        dn = dist / diag
        sigma = (1.0 - dn) * SIGMA_MIN + dn * SIGMA_MAX
        inv2s2 = 1.0 / (2.0 * sigma * sigma)
        base = -dist * np.sqrt(sigma) / (math.pi * sigma**4)
        b2 = base * inv2s2

        def tiles(a):
            t = a.reshape(N_STRIPS, S_ROWS, N_BLOCKS, S_COLS)
            return t.transpose(0, 2, 1, 3).reshape(128, S_ROWS, S_COLS)

        wv = np.empty((128, NV, S_ROWS, S_COLS), dtype=bf)
        wv[:, 0] = tiles(base).astype(bf)
        bt, b2t = tiles(base), tiles(b2)
        for i, v in enumerate(V_ORD):
            wv[:, 1 + i] = (bt - v * b2t).astype(bf)
        INV = np.ascontiguousarray(tiles(inv2s2).astype(np.float32))

        in_maps.append(
            {"xp": XP, "xp1": XP1, "wv": np.ascontiguousarray(wv), "inv": INV}
        )
    return in_maps


def _gather(results):
    out = np.empty((B, C, H, W), dtype=np.float32)
    for core in range(N_CORES):
        b, half = divmod(core, 2)
        y0 = half * 128
        o = results[core]["out"].astype(np.float32)
        o = o.reshape(N_STRIPS, N_BLOCKS, C, S_ROWS, S_COLS)
        o = o.transpose(2, 0, 3, 1, 4).reshape(C, 128, W)
        out[b, :, y0 : y0 + 128, :] = o
    return out


def kernel(x, foa_xy, _trace=False, _tmpdir=None):
    import sys

    if "/opt/trn_rl_repo" not in sys.path:
        sys.path.insert(0, "/opt/trn_rl_repo")
    from concourse.bass_utils import run_bass_kernel_spmd

    nc = _get_program()
    in_maps = _host_prep(np.asarray(x), np.asarray(foa_xy))
    kw = {}
    if _trace:
        kw = dict(trace=True, trace_cores=[], tmpdir=_tmpdir)
    res = run_bass_kernel_spmd(nc, in_maps, list(range(N_CORES)), **kw)
    out = _gather(res.results)
    if _trace:
        return out, res
    return out
